# revision 1
# baseline (speedup 1.0000x reference)
"""Sliding-window causal GQA self-attention kernel for 8 Trainium2 NeuronCores.

Sharding: core c -> (batch b = c//4, kv-head g = c%4, q-heads 4g..4g+3).
Each core computes its 4 q-heads' attention and a partial output projection
(y_heads @ Wo[rows]); the host sums the 4 partials per batch.

On-chip layout is feature-major ("transposed"): activations live as
[features, tokens] tiles so every matmul contracts over the partition dim.
Scores are computed k-major (sT[k, q]); softmax needs no max-subtraction
because q/k are rms-normalized (|score| <= 8). The softmax denominator is
produced by an extra ones-column appended to V in the PV matmul. Causal and
sliding-window masks are added in PSUM by triangular-counting matmuls.
All matmuls use the float32r dtype view (full-rate fp32 at free-dim >= 256).
"""

import numpy as np

import concourse.bass as bass
import concourse.mybir as mybir
import concourse.tile as tile
from concourse.bass import ds, ts

F32 = mybir.dt.float32
F32R = mybir.dt.float32r
AF = mybir.ActivationFunctionType

B, T, NE = 2, 2048, 1024
NH, NKV, HD = 16, 4, 64
GC = 32
WIN = 1024
EPS = 1e-6
BIG = 1.0e9
NCORES = 8
QB = 256          # q-block (moving free dim of QK/PV matmuls)
NQB = T // QB     # 8
NKB = T // 128    # 16 k-blocks
SCALE = 1.0 / 8.0  # 1/sqrt(HD)


def _r(x):
    return x.bitcast(F32R)


def _dma_r(nc, dst, src):
    nc.sync.dma_start(_r(dst), _r(src))


def _build_nc():
    nc = bass.Bass(trn_type="TRN2", target_bir_lowering=False)

    d = {}
    for name, shape in [
        ("xT", (NE, T)), ("ve", (T, HD)),
        ("cos4", (128, T)), ("sin4", (128, T)),
        ("coskv", (128, T)), ("sinkv", (128, T)),
        ("wq", (NE, 256)), ("wkv", (NE, 128)), ("wg", (GC, 1)),
        ("wo", (256, NE)),
        ("pswq", (128, 128)), ("pswkv", (128, 128)),
        ("bdq", (128, 2)), ("bdk", (128, 1)),
        ("e2sel", (2, 128)), ("ones64", (1, 64)), ("ident", (128, 128)),
        ("triA", (128, 128)), ("triA2", (128, 128)),
        ("bc0", (128, QB)), ("bc1", (128, QB)),
        ("bw0", (128, QB)), ("bw1", (128, QB)),
        ("onesrow", (1, T)), ("onescol", (128, 1)),
    ]:
        d[name] = nc.dram_tensor(name, list(shape), F32, kind="ExternalInput")
    out_d = nc.dram_tensor("out", [T, NE], F32, kind="ExternalOutput")

    with tile.TileContext(nc) as tc:
        with (
            nc.allow_low_precision(reason="float32r views of fp32 data"),
            tc.tile_pool(name="persist", bufs=1) as pp,
            tc.tile_pool(name="smalls", bufs=4) as sm,
        ):
            # ---- persistent tiles ----
            qhat = [pp.tile([128, T], F32, tag=f"qhat{i}", name=f"qhat{i}") for i in range(2)]
            kdup = pp.tile([128, T], F32, tag="kdup")
            vaug = [pp.tile([128, HD + 1], F32, tag=f"vaug{k}", name=f"vaug{k}")
                    for k in range(NKB)]
            cst = {}
            for nm, shp in [("e2sel", [2, 128]), ("ident", [128, 128])]:
                cst[nm] = pp.tile(shp, F32, tag=nm, name=nm)
            _dma_r(nc, cst["e2sel"][:], d["e2sel"][:])
            nc.sync.dma_start(cst["ident"][:], d["ident"][:])
            eps_sb = pp.tile([128, 1], F32, tag="eps")
            nc.vector.memset(eps_sb[:], EPS)

            # =================================================================
            # Phase A: projections + rope + rmsnorm + vaug build
            # =================================================================
            with (
                tc.tile_pool(name="xp", bufs=1) as xp,
                tc.tile_pool(name="work", bufs=1) as wk,
                tc.tile_pool(name="trig", bufs=1) as trg,
                tc.tile_pool(name="pj_ps", bufs=2, space="PSUM") as pjp,
                tc.tile_pool(name="sw_ps", bufs=2, space="PSUM") as swp,
                tc.tile_pool(name="aux_ps", bufs=1, space="PSUM") as axp,
            ):
                xsb = [xp.tile([128, T], F32, tag=f"x{e}", name=f"x{e}") for e in range(8)]
                wq_sb = [xp.tile([128, 256], F32, tag=f"wq{e}", name=f"wqs{e}")
                         for e in range(8)]
                wkv_sb = [xp.tile([128, 128], F32, tag=f"wkv{e}", name=f"wkvs{e}")
                          for e in range(8)]
                for e in range(8):
                    _dma_r(nc, xsb[e][:], d["xT"][ds(128 * e, 128), :])
                    _dma_r(nc, wq_sb[e][:], d["wq"][ds(128 * e, 128), :])
                    _dma_r(nc, wkv_sb[e][:], d["wkv"][ds(128 * e, 128), :])
                wg_sb = sm.tile([GC, 1], F32, tag="wg")
                _dma_r(nc, wg_sb[:], d["wg"][:])
                ve_sb = xp.tile([128, NKB, HD], F32, tag="ve")
                nc.sync.dma_start(
                    ve_sb[:], d["ve"][:, :].rearrange("(n p) d -> p n d", p=128))
                aux = {}
                for nm, shp in [("pswq", [128, 128]), ("pswkv", [128, 128]),
                                ("bdq", [128, 2]), ("bdk", [128, 1])]:
                    aux[nm] = xp.tile(shp, F32, tag=nm, name=f"aux_{nm}")
                    _dma_r(nc, aux[nm][:], d[nm][:])

                # gate: u = x[:, :GC] @ wg ; g2 = 2*sigmoid(u) token-major
                gate_ps = axp.tile([128, NKB], F32, tag="aux")
                for kb in range(NKB):
                    nc.tensor.matmul(
                        gate_ps[:, ds(kb, 1)],
                        xsb[0][0:GC, ts(kb, 128)], wg_sb[:],
                        start=True, stop=True)
                g2 = xp.tile([128, NKB], F32, tag="g2")
                nc.scalar.activation(g2[:], gate_ps[:], AF.Exp, scale=-1.0)
                nc.vector.tensor_scalar_add(g2[:], g2[:], 1.0)
                nc.vector.reciprocal(g2[:], g2[:])
                nc.scalar.mul(g2[:], g2[:], 2.0)

                def project_rope(widx, w_tiles, mcols, psw, cos_t, sin_t,
                                 bd, nh):
                    """x @ W -> [128, T] feature-major tile; rope; rms stats.
                    Returns (roped tile, rs [nh rows used, T] rsqrt scales)."""
                    raw = wk.tile([128, T], F32, tag="w0", bufs=2)
                    t1 = wk.tile([128, T], F32, tag="w1")
                    tmp2 = wk.tile([128, T], F32, tag="w2")
                    for nchk in range(4):
                        cols = ds(512 * nchk, 512)
                        ps = pjp.tile([128, 512], F32, tag="pj")
                        for e in range(8):
                            nc.tensor.matmul(
                                ps[:], _r(w_tiles[e][:, mcols]),
                                _r(xsb[e][:, cols]),
                                start=(e == 0), stop=(e == 7))
                        nc.any.tensor_copy(_r(raw[:, cols]), ps[:])
                    # rope: roped = raw*cos + (psw @ raw)*sin   (in place: raw)
                    nc.vector.tensor_mul(_r(t1[:]), raw[:], cos_t[:])
                    for nchk in range(4):
                        cols = ds(512 * nchk, 512)
                        sw = swp.tile([128, 512], F32, tag="sw")
                        nc.tensor.matmul(sw[:], _r(psw[:]), _r(raw[:, cols]),
                                         start=True, stop=True)
                        nc.vector.tensor_mul(tmp2[:, cols], sw[:],
                                             sin_t[:, cols])
                    roped = raw
                    nc.vector.tensor_add(_r(roped[:]), t1[:], tmp2[:])
                    # rms stats: ms[h, t] = sum_f-in-head roped^2
                    sq = t1
                    nc.vector.tensor_mul(_r(sq[:]), roped[:], roped[:])
                    msps = axp.tile([nh, T], F32, tag="aux")
                    for nchk in range(4):
                        cols = ds(512 * nchk, 512)
                        nc.tensor.matmul(msps[:, cols], _r(bd[:, 0:nh]),
                                         _r(sq[:, cols]), start=True, stop=True)
                    lnm = sm.tile([2, T], F32, tag="lnm", bufs=1,
                                  name=f"lnm{widx}")
                    nc.scalar.activation(lnm[0:nh, :], msps[:], AF.Ln,
                                         scale=1.0 / HD, bias=eps_sb[0:nh, :])
                    rs = sm.tile([2, T], F32, tag="rs", bufs=2,
                                 name=f"rs{widx}")
                    if nh == 1:
                        _dma_r(nc, rs[0:2, :],
                               d["onesrow"][:, :].to_broadcast([2, T]))
                    nc.scalar.activation(_r(rs[0:nh, :]), lnm[0:nh, :], AF.Exp,
                                         scale=-0.5)
                    return roped, rs

                # q heads: two tiles of 2 heads each
                cos_q = trg.tile([128, T], F32, tag="tc")
                sin_q = trg.tile([128, T], F32, tag="tsn")
                nc.sync.dma_start(cos_q[:], d["cos4"][:])
                nc.sync.dma_start(sin_q[:], d["sin4"][:])
                for i in range(2):
                    roped, rs = project_rope(i, wq_sb, ds(128 * i, 128),
                                             aux["pswq"], cos_q, sin_q,
                                             aux["bdq"], 2)
                    for nchk in range(4):
                        cols = ds(512 * nchk, 512)
                        rsb = swp.tile([128, 512], F32, tag="sw")
                        nc.tensor.matmul(rsb[:], _r(cst["e2sel"][:]),
                                         _r(rs[0:2, cols]),
                                         start=True, stop=True)
                        nc.vector.tensor_mul(_r(qhat[i][:, cols]),
                                             roped[:, cols], rsb[:])

                # kv tile: k rows 0:64 roped+rms'd, v rows 64:128 passthrough
                cos_kv = trg.tile([128, T], F32, tag="tc")
                sin_kv = trg.tile([128, T], F32, tag="tsn")
                nc.sync.dma_start(cos_kv[:], d["coskv"][:])
                nc.sync.dma_start(sin_kv[:], d["sinkv"][:])
                ropedkv, rsk = project_rope(2, wkv_sb, ds(0, 128),
                                            aux["pswkv"], cos_kv, sin_kv,
                                            aux["bdk"], 1)
                kvfin = wk.tile([128, T], F32, tag="w2")
                for nchk in range(4):
                    cols = ds(512 * nchk, 512)
                    rsb = swp.tile([128, 512], F32, tag="sw")
                    nc.tensor.matmul(rsb[:], _r(cst["e2sel"][:]),
                                     _r(rsk[0:2, cols]), start=True, stop=True)
                    nc.vector.tensor_mul(_r(kvfin[:, cols]), ropedkv[:, cols],
                                         rsb[:])
                # kdup: k-hat on both partition halves (for head pairing)
                nc.any.tensor_copy(_r(kdup[0:64, :]), kvfin[0:64, :])
                nc.any.tensor_copy(_r(kdup[64:128, :]), kvfin[0:64, :])
                # vaug: token-major v (+ gate*ve), ones column appended
                for kb in range(NKB):
                    vt = pjp.tile([128, HD], F32, tag="pj")
                    nc.tensor.transpose(vt[:], kvfin[64:128, ts(kb, 128)],
                                        cst["ident"][64:128, 64:128])
                    gv = sm.tile([128, HD], F32, tag="gv")
                    nc.vector.tensor_scalar_mul(gv[:], ve_sb[:, kb, :],
                                                g2[:, ds(kb, 1)])
                    _dma_r(nc, vaug[kb][:, HD:HD + 1], d["onescol"][:])
                    nc.vector.tensor_add(_r(vaug[kb][:, 0:HD]), gv[:], vt[:])

            # =================================================================
            # Phase B: attention + output projection
            # =================================================================
            with (
                tc.tile_pool(name="pb", bufs=1) as pb,
                tc.tile_pool(name="sc_ps", bufs=5, space="PSUM") as scp,
                tc.tile_pool(name="yt_ps", bufs=2, space="PSUM") as ytp,
                tc.tile_pool(name="rb_ps", bufs=1, space="PSUM") as rbp,
                tc.tile_pool(name="et", bufs=6) as etp,
                tc.tile_pool(name="stage", bufs=8) as stg,
            ):
                ytall = [pb.tile([128, T], F32, tag=f"ytall{i}",
                                 name=f"ytall{i}") for i in range(2)]
                wo_sb = [[pb.tile([128, 512], F32, tag=f"wo{i}{n}",
                                  name=f"wo{i}{n}")
                          for n in range(2)] for i in range(2)]
                for nm, shp in [("ones64", [1, 64]), ("triA", [128, 128]),
                                ("triA2", [128, 128]), ("bc0", [128, QB]),
                                ("bc1", [128, QB]), ("bw0", [128, QB]),
                                ("bw1", [128, QB])]:
                    cst[nm] = pb.tile(shp, F32, tag=nm, name=f"pb_{nm}")
                    _dma_r(nc, cst[nm][:], d[nm][:])
                for i in range(2):
                    for n in range(2):
                        _dma_r(nc, wo_sb[i][n][:],
                               d["wo"][ds(128 * i, 128), ds(512 * n, 512)])
                for qb in range(NQB):
                    kbs = list(range(max(0, 2 * qb - 8), 2 * qb + 2))
                    qsl = ds(QB * qb, QB)
                    for pair in range(2):
                        yts = [ytp.tile([HD + 1, QB], F32, tag="yt", name=f"yt{qb}_{pair}_{_}")
                               for _ in range(2)]
                        groups = [kbs[i:i + 2] for i in range(0, len(kbs), 2)]
                        for grp in groups:
                            w = QB * len(grp)
                            scs = [scp.tile([128, w], F32, tag="score", name=f"sc{qb}_{pair}_{grp[0]}_{_}")
                                   for _ in range(2)]
                            for ki, kb in enumerate(grp):
                                cols = ds(QB * ki, QB)
                                if kb == 2 * qb:
                                    mask = (cst["triA"], cst["bc0"])
                                elif kb == 2 * qb + 1:
                                    mask = (cst["triA"], cst["bc1"])
                                elif kb == 2 * qb - 8:
                                    mask = (cst["triA2"], cst["bw0"])
                                elif kb == 2 * qb - 7:
                                    mask = (cst["triA2"], cst["bw1"])
                                else:
                                    mask = None
                                for hl in range(2):
                                    rows = ds(64 * hl, 64)
                                    nc.tensor.matmul(
                                        scs[hl][:, cols],
                                        _r(kdup[rows, ts(kb, 128)]),
                                        _r(qhat[pair][rows, qsl]),
                                        start=True, stop=(mask is None))
                                    if mask is not None:
                                        nc.tensor.matmul(
                                            scs[hl][:, cols],
                                            _r(mask[0][:]), _r(mask[1][:]),
                                            start=False, stop=True)
                            ets = [etp.tile([128, w], F32, tag="et", name=f"et{qb}_{pair}_{grp[0]}_{_}")
                                   for _ in range(2)]
                            for hl in range(2):
                                nc.scalar.activation(_r(ets[hl][:]),
                                                     scs[hl][:],
                                                     AF.Exp, scale=SCALE)
                                for ki, kb in enumerate(grp):
                                    cols = ds(QB * ki, QB)
                                    nc.tensor.matmul(
                                        yts[hl][:], _r(vaug[kb][:]),
                                        _r(ets[hl][:, cols]),
                                        start=(kb == kbs[0]),
                                        stop=(kb == kbs[-1]))
                        for hl in range(2):
                            rinv = sm.tile([1, QB], F32, tag="rinv")
                            nc.vector.reciprocal(_r(rinv[:]),
                                                 yts[hl][HD:HD + 1, :])
                            rb = rbp.tile([128, 512], F32, tag="rbpo")
                            nc.tensor.matmul(rb[0:64, 0:QB],
                                             _r(cst["ones64"][:]), _r(rinv[:]),
                                             start=True, stop=True)
                            ysb = stg.tile([64, QB], F32, tag="ysb")
                            nc.any.tensor_copy(ysb[:], yts[hl][0:HD, :])
                            nc.vector.tensor_mul(
                                _r(ytall[pair][ds(64 * hl, 64), qsl]),
                                ysb[:], rb[0:64, 0:QB])
                    # output projection for the two finished token tiles
                    for tt in (2 * qb, 2 * qb + 1):
                        for nn in range(2):
                            po = rbp.tile([128, 512], F32, tag="rbpo")
                            for i in range(2):
                                nc.tensor.matmul(
                                    po[:], _r(ytall[i][:, ts(tt, 128)]),
                                    _r(wo_sb[i][nn][:]),
                                    start=(i == 0), stop=(i == 1))
                            osb = stg.tile([128, 512], F32, tag="osb")
                            nc.any.tensor_copy(osb[:], po[:])
                            nc.sync.dma_start(
                                out_d[ts(tt, 128), ds(512 * nn, 512)], osb[:])

    return nc


# ---------------------------------------------------------------------------
# walrus workaround: this build rejects >1 sync-wait on CTRL-class ops
# (e.g. the Tile tail Drain). Move excess waits onto NOPs inserted before.
# ---------------------------------------------------------------------------
_CTRL_TYPES = (mybir.InstDrain, mybir.InstNoOp, mybir.InstEventSemaphore)


def _split_excess_waits(nc, limit=1):
    for fn in nc.m.functions:
        for bb in fn.blocks:
            out, changed = [], False
            for inst in bb.instructions:
                si = inst.sync_info
                waits = list(si.on_wait) if si is not None and si.on_wait else []
                if len(waits) > limit:
                    extra, keep = waits[:-limit], waits[-limit:]
                    while extra:
                        chunk, extra = extra[:limit], extra[limit:]
                        nop = mybir.InstNoOp(
                            name=f"{inst.name}-wsplit{len(out)}", ins=[],
                            outs=[])
                        nop.engine = inst.engine
                        nop.sync_info = mybir.SyncInfo(on_wait=chunk,
                                                       on_update=[])
                        out.append(nop)
                    si.on_wait = keep
                    inst.sync_info = si
                    changed = True
                out.append(inst)
            if changed:
                bb.instructions = out


# ---------------------------------------------------------------------------
# Host-side constants (shared by all cores)
# ---------------------------------------------------------------------------
def _host_constants():
    c = {}
    m = np.arange(128)[:, None]
    j = np.arange(128)[None, :]
    i = np.arange(QB)[None, :]
    c["triA"] = (m <= j).astype(np.float32)          # causal counting lhsT
    c["triA2"] = (m >= j).astype(np.float32)         # window counting lhsT
    c["bc0"] = np.where(m > i, -BIG, 0.0).astype(np.float32)
    c["bc1"] = np.where(m > i - 128, -BIG, 0.0).astype(np.float32)
    c["bw0"] = np.where(m < i, -BIG, 0.0).astype(np.float32)
    c["bw1"] = np.where(m + 128 < i, -BIG, 0.0).astype(np.float32)
    sw = np.zeros((128, 128), np.float32)            # pswq[f, m]=1 iff f=sig(m)
    for mm in range(128):
        f = mm + 32 if (mm % 64) < 32 else mm - 32
        sw[f, mm] = 1.0
    c["pswq"] = sw
    swkv = sw.copy()
    swkv[:, 64:] = 0.0
    c["pswkv"] = swkv
    bdq = np.zeros((128, 2), np.float32)
    bdq[0:64, 0] = 1.0
    bdq[64:128, 1] = 1.0
    c["bdq"] = bdq
    bdk = np.zeros((128, 1), np.float32)
    bdk[0:64, 0] = 1.0
    c["bdk"] = bdk
    e2 = np.zeros((2, 128), np.float32)
    e2[0, 0:64] = 1.0
    e2[1, 64:128] = 1.0
    c["e2sel"] = e2
    c["ones64"] = np.ones((1, 64), np.float32)
    c["ident"] = np.eye(128, dtype=np.float32)
    c["onesrow"] = np.ones((1, T), np.float32)
    c["onescol"] = np.ones((128, 1), np.float32)
    return c


def _trig(cos_b, sin_b):
    """cos_b/sin_b: [T, HD//2] -> the four [128, T] rope coefficient maps."""
    ct = np.ascontiguousarray(cos_b.T)               # [32, T]
    st = np.ascontiguousarray(sin_b.T)
    cos4 = np.tile(ct, (4, 1)).astype(np.float32)    # [c;c;c;c]
    sin4 = np.tile(np.concatenate([st, -st], 0), (2, 1)).astype(np.float32)
    coskv = np.concatenate([ct, ct, np.ones((64, T), np.float32)], 0)
    sinkv = np.concatenate([st, -st, np.zeros((64, T), np.float32)], 0)
    return cos4, sin4, coskv.astype(np.float32), sinkv.astype(np.float32)


# ---------------------------------------------------------------------------
# Cached PJRT runner (compile once per process)
# ---------------------------------------------------------------------------
_RUNNER = None


def _get_runner():
    global _RUNNER
    if _RUNNER is not None:
        return _RUNNER
    import jax
    from jax.experimental.shard_map import shard_map
    from jax.sharding import Mesh, PartitionSpec
    from concourse.bass2jax import (_bass_exec_p, install_neuronx_cc_hook,
                                    partition_id_tensor)

    nc = _build_nc()
    _split_excess_waits(nc)
    install_neuronx_cc_hook()

    pid_name = (nc.partition_id_tensor.name
                if nc.partition_id_tensor is not None else None)
    in_names, out_names, out_avals, zero_outs = [], [], [], []
    for alloc in nc.m.functions[0].allocations:
        if not isinstance(alloc, mybir.MemoryLocationSet):
            continue
        name = alloc.memorylocations[0].name
        if alloc.kind == "ExternalInput":
            if name == pid_name:
                continue
            in_names.append(name)
        elif alloc.kind == "ExternalOutput":
            np_dt = mybir.dt.np(alloc.dtype)
            out_names.append(name)
            out_avals.append(
                jax.core.ShapedArray(tuple(alloc.tensor_shape), np_dt))
            zero_outs.append(
                np.zeros(tuple(alloc.tensor_shape), np_dt))

    def _body(*args):
        operands = list(args)
        if pid_name is not None:
            operands.append(partition_id_tensor())
        outs = _bass_exec_p.bind(
            *operands,
            out_avals=tuple(out_avals),
            in_names=(tuple(in_names) + tuple(out_names)
                      + ((pid_name,) if pid_name else ())),
            out_names=tuple(out_names),
            lowering_input_output_aliases=(),
            sim_require_finite=True,
            sim_require_nnan=True,
            nc=nc,
        )
        return tuple(outs)

    devices = jax.devices()[:NCORES]
    mesh = Mesh(np.asarray(devices), ("core",))
    n_args = len(in_names) + len(out_names)
    sharded = jax.jit(
        shard_map(_body, mesh=mesh,
                  in_specs=(PartitionSpec("core"),) * n_args,
                  out_specs=(PartitionSpec("core"),) * len(out_names),
                  check_rep=False),
        keep_unused=True,
    )

    def run(in_maps):
        concat_in = [
            np.concatenate([in_maps[c][nm] for c in range(NCORES)], axis=0)
            for nm in in_names
        ]
        concat_zero = [
            np.zeros((NCORES * z.shape[0], *z.shape[1:]), z.dtype)
            for z in zero_outs
        ]
        outs = sharded(*concat_in, *concat_zero)
        res = []
        for c in range(NCORES):
            res.append({
                nm: np.asarray(outs[i]).reshape(NCORES, *out_avals[i].shape)[c]
                for i, nm in enumerate(out_names)
            })
        return res

    _RUNNER = {"run": run, "sharded": sharded, "in_names": in_names,
               "out_names": out_names, "out_avals": out_avals,
               "zero_outs": zero_outs, "nc": nc, "mesh": mesh}
    return _RUNNER


def _make_in_maps(x, ve, cos, sin, Wq, Wk, Wv, Wo, Wg):
    cstc = _host_constants()
    in_maps = []
    for c in range(NCORES):
        b, g = c // 4, c % 4
        cos4, sin4, coskv, sinkv = _trig(np.asarray(cos[b]),
                                         np.asarray(sin[b]))
        m = {
            "xT": np.ascontiguousarray(np.asarray(x[b]).T),
            "ve": np.ascontiguousarray(np.asarray(ve[b])[:, HD * g:HD * (g + 1)]),
            "cos4": cos4, "sin4": sin4, "coskv": coskv, "sinkv": sinkv,
            "wq": np.ascontiguousarray(Wq[:, 256 * g:256 * (g + 1)]),
            "wkv": np.ascontiguousarray(
                np.concatenate([Wk[:, HD * g:HD * (g + 1)],
                                Wv[:, HD * g:HD * (g + 1)]], axis=1)),
            "wg": np.ascontiguousarray(Wg[:, g:g + 1]),
            "wo": np.ascontiguousarray(Wo[256 * g:256 * (g + 1), :]),
        }
        m.update(cstc)
        in_maps.append({k: np.asarray(v, np.float32) for k, v in m.items()})
    return in_maps


def kernel(x, ve, cos, sin, Wq, Wk, Wv, Wo, Wg, window_size):
    assert int(window_size) == WIN, f"kernel hardcodes window={WIN}"
    x, ve, cos, sin = (np.asarray(a, np.float32) for a in (x, ve, cos, sin))
    Wq, Wk, Wv, Wo, Wg = (np.asarray(a, np.float32)
                          for a in (Wq, Wk, Wv, Wo, Wg))
    runner = _get_runner()
    in_maps = _make_in_maps(x, ve, cos, sin, Wq, Wk, Wv, Wo, Wg)
    res = runner["run"](in_maps)
    out = np.zeros((B, T, NE), np.float32)
    for c in range(NCORES):
        out[c // 4] += res[c]["out"]
    return out



# revision 2
# speedup vs baseline: 144.3319x; 144.3319x over previous
"""Sliding-window causal GQA self-attention kernel for 8 Trainium2 NeuronCores.

v2: engine-balanced rewrite of the baseline.
Sharding: core c -> (batch b = c//4, kv-head g = c%4, q-heads 4g..4g+3).
Each core computes its 4 q-heads' attention and a partial output projection
(y_heads @ Wo[rows]); the host sums the 4 partials per batch.

Changes vs baseline (guided by the CoreSim cost model):
- inputs packed into 2 dram tensors (xT + flat aux) + 1 output -> 3 PJRT
  buffers instead of 35 (each buffer costs ~0.45 ms of axon dispatch wall).
- rmsnorm rsqrt via Ln + Exp(-0.5x) (walrus has no Dsqrt/Rsqrt).
- gate 2*sigmoid via one Sigmoid activation (ve pre-scaled by 2 on host).
- rope sign folded into the signed shuffle matrix psw; trig maps built by
  replicated DMA reads of the 32-row cos/sin tables (2MB instead of 4MB).
- rms/softmax scales broadcast across partitions with tiny K=1
  ones-matmuls into PSUM (walrus rejects GpSimd PartitionBroadcast);
  the following elementwise multiplies read PSUM directly.
- softmax normalization reads PV PSUM directly (no staging copy).
- diag/window mask pairs applied with one [128,512] counting matmul.
- output staged bf16 (halves out DMA); host upcasts and sums partials.
"""

import numpy as np

import concourse.bass as bass
import concourse.mybir as mybir
import concourse.tile as tile
from concourse.bass import ds, ts

F32 = mybir.dt.float32
BF16 = mybir.dt.bfloat16
F32R = mybir.dt.float32r
AF = mybir.ActivationFunctionType

B, T, NE = 2, 2048, 1024
NH, NKV, HD = 16, 4, 64
GC = 32
WIN = 1024
EPS = 1e-6
BIG = 1.0e9
NCORES = 8
QB = 256          # q-block (moving free dim of QK/PV matmuls)
NQB = T // QB     # 8
NKB = T // 128    # 16 k-blocks
SCALE = 1.0 / 8.0  # 1/sqrt(HD)

# ---- flat aux layout: name -> (shape, offset) filled below ----
_AUX_REGIONS = [
    ("ct", (GC, T)), ("st", (GC, T)),
    ("ve2", (T * HD,)),              # host pre-multiplied by 2.0
    ("wo", (256, NE)),
    ("psw", (128, 128)), ("bd", (128, 2)), ("ones64", (1, 64)),
    ("e2sel", (2, 128)),
    ("bc01", (128, 2 * QB)), ("bw01", (128, 2 * QB)),
    ("triA", (128, 128)), ("triA2", (128, 128)),
    ("ident", (128, 128)),
]
_AUX_OFF = {}
_off = 0
for _nm, _shp in _AUX_REGIONS:
    _AUX_OFF[_nm] = _off
    _off += int(np.prod(_shp))
AUX_LEN = _off

# bf16 flat input: projection weights (x rides its own bf16 tensor)
_AUXH_REGIONS = [("wq", (NE, 256)), ("wkv", (NE, 128)), ("wg", (GC, 1))]
_AUXH_OFF = {}
_off = 0
for _nm, _shp in _AUXH_REGIONS:
    _AUXH_OFF[_nm] = _off
    _off += int(np.prod(_shp))
AUXH_LEN = _off


def _r(x):
    return x.bitcast(F32R)


def _build_nc(nrep=1):
    nc = bass.Bass(trn_type="TRN2", target_bir_lowering=False)

    xT_d = nc.dram_tensor("xT", [NE, T], BF16, kind="ExternalInput")
    aux_d = nc.dram_tensor("aux", [AUX_LEN], F32, kind="ExternalInput")
    auxh_d = nc.dram_tensor("auxh", [AUXH_LEN], BF16, kind="ExternalInput")
    out_d = nc.dram_tensor("out", [T, NE], BF16, kind="ExternalOutput")

    def aux(nm, p, n):
        """[p, n] view of flat aux region nm."""
        o = _AUX_OFF[nm]
        return aux_d[ds(o, p * n)].rearrange("(p n) -> p n", p=p)

    with tile.TileContext(nc) as tc:
        with (
            nc.allow_low_precision(reason="float32r views; bf16 output"),
            tc.tile_pool(name="persist", bufs=1) as pp,
            tc.tile_pool(name="attbuf", bufs=2) as attp,
            tc.tile_pool(name="smalls", bufs=4) as sm,
        ):
            # ---- persistent constants ----
            cst = {}
            for nm, shp in [("psw", [128, 128]), ("bd", [128, 2]),
                            ("ones64", [1, 64]), ("e2sel", [2, 128]),
                            ("bc01", [128, 2 * QB]), ("bw01", [128, 2 * QB]),
                            ("triA", [128, 128]), ("triA2", [128, 128]),
                            ("ident", [128, 128])]:
                cst[nm] = pp.tile(shp, F32, tag=nm, name=f"cst_{nm}")
                nc.sync.dma_start(_r(cst[nm][:]), _r(aux(nm, *shp)))
            cst["wg"] = pp.tile([GC, 1], BF16, tag="wg", name="cst_wg")
            nc.sync.dma_start(
                cst["wg"][:],
                auxh_d[ds(_AUXH_OFF["wg"], GC)].rearrange("(p n) -> p n", p=GC))
            eps_sb = pp.tile([128, 1], F32, tag="eps")
            nc.vector.memset(eps_sb[:], EPS)
            cst["eps"] = eps_sb
            wo_sb = [[pp.tile([128, 512], F32, tag=f"wo{i}{n}", name=f"wo{i}{n}")
                      for n in range(2)] for i in range(2)]
            for i in range(2):
                for n in range(2):
                    nc.sync.dma_start(
                        _r(wo_sb[i][n][:]),
                        _r(aux_d[ds(_AUX_OFF["wo"] + 128 * i * NE, 128 * NE)]
                           .rearrange("(p n) -> p n", p=128)[:, ds(512 * n, 512)]))

            loads = [_load_inputs(nc, tc, xT_d, aux_d, auxh_d, aux, 0,
                                   split_queues=True)]
            for rep in range(nrep):
                _one_rep(nc, tc, attp, sm, aux, out_d, cst, wo_sb, rep,
                         loads[rep],
                         prefetch=(lambda r: loads.append(_load_inputs(
                             nc, tc, xT_d, aux_d, auxh_d, aux, r)))
                         if rep + 1 < nrep else None)
    return nc


def _load_inputs(nc, tc, xT_d, aux_d, auxh_d, aux, rep, split_queues=False):
    """Input DMAs for one rep. All on the SP queue so they are never
    stuck behind output DMAs (Pool queue); rep 0 splits across both
    queues since no outputs are pending yet."""
    xp = tc.alloc_tile_pool(name=f"xp{rep}", bufs=1)
    trg = tc.alloc_tile_pool(name=f"trig{rep}", bufs=1)
    d = {"xp": xp, "trg": trg}
    dq2 = nc.gpsimd if split_queues else nc.sync
    d["xsb"] = [xp.tile([128, T], BF16, tag=f"x{e}", name=f"x{e}_{rep}")
                for e in range(8)]
    d["wq_sb"] = [xp.tile([128, 256], BF16, tag=f"wq{e}", name=f"wqs{e}_{rep}")
                  for e in range(8)]
    d["wkv_sb"] = [xp.tile([128, 128], BF16, tag=f"wkv{e}",
                           name=f"wkvs{e}_{rep}")
                   for e in range(8)]
    for e in range(8):
        dq = dq2 if e % 2 else nc.sync
        dq.dma_start(d["xsb"][e][:], xT_d[ds(128 * e, 128), :])
        nc.sync.dma_start(
            d["wkv_sb"][e][:],
            auxh_d[ds(_AUXH_OFF["wkv"] + 128 * e * 128, 128 * 128)]
            .rearrange("(p n) -> p n", p=128))
        dq.dma_start(
            d["wq_sb"][e][:],
            auxh_d[ds(_AUXH_OFF["wq"] + 128 * e * 256, 128 * 256)]
            .rearrange("(p n) -> p n", p=128))
    d["ve_sb"] = xp.tile([128, NKB, HD], F32, tag="ve", name=f"ve_{rep}")
    nc.sync.dma_start(
        d["ve_sb"][:],
        aux_d[ds(_AUX_OFF["ve2"], T * HD)]
        .rearrange("(n p d) -> p n d", p=128, d=HD))
    # trig maps: 4x replicated 32-row tables, sign carried by psw
    d["cos4"] = trg.tile([128, T], F32, tag="tc", name=f"tc_{rep}")
    d["sin4"] = trg.tile([128, T], F32, tag="tsn", name=f"tsn_{rep}")
    for q in range(4):
        dq = dq2 if q % 2 else nc.sync
        dq.dma_start(d["cos4"][ds(32 * q, 32), :], aux("ct", GC, T))
        dq.dma_start(d["sin4"][ds(32 * q, 32), :], aux("st", GC, T))
    return d


def _one_rep(nc, tc, attp, sm, aux, out_d, cst, wo_sb, rep, ld,
             prefetch=None):
    # cross-rep double-buffered attention tiles (tag reuse rotates bufs)
    qhat = [attp.tile([128, T], F32, tag=f"qhat{i}", name=f"qhat{i}_{rep}")
            for i in range(2)]
    kdup = attp.tile([128, T], F32, tag="kdup", name=f"kdup_{rep}")
    vaug = [attp.tile([128, HD + 1], F32, tag=f"vaug{k}",
                      name=f"vaug{k}_{rep}")
            for k in range(NKB)]
    # =================================================================
    # Phase A: projections + rope + rmsnorm + vaug build
    # =================================================================
    xsb, wq_sb, wkv_sb = ld["xsb"], ld["wq_sb"], ld["wkv_sb"]
    ve_sb, cos4, sin4 = ld["ve_sb"], ld["cos4"], ld["sin4"]
    with (
        tc.tile_pool(name=f"work{rep}", bufs=1) as wk,
        tc.tile_pool(name=f"pj_ps{rep}", bufs=2, space="PSUM") as pjp,
        tc.tile_pool(name=f"sw_ps{rep}", bufs=2, space="PSUM") as swp,
        tc.tile_pool(name=f"aux_ps{rep}", bufs=1, space="PSUM") as axp,
    ):
        # gate: u[t, kb] = x[0:GC, t] @ wg ; g2 = sigmoid(u) (ve carries 2x)
        gate_ps = axp.tile([128, NKB], F32, tag="aux")
        for kb in range(NKB):
            nc.tensor.matmul(
                gate_ps[:, ds(kb, 1)],
                xsb[0][0:GC, ts(kb, 128)], cst["wg"][:],
                start=True, stop=True)
        g2 = ld["xp"].tile([128, NKB], F32, tag="g2", name=f"g2_{rep}")
        nc.scalar.activation(g2[:], gate_ps[:], AF.Sigmoid)

        def project_rope(widx, w_tiles, mcols, nh):
            """x @ W -> [128, T] feature-major roped tile + rms scales.
            Chunked at 512 cols so PE/Pool/DVE stages pipeline.
            Returns (roped [128,T], [nh x [1,T]] rs tiles)."""
            nr = 64 * nh  # rows that get rope+rms (kv: k rows only)
            raw = wk.tile([128, T], F32, tag="w0", bufs=2, name=f"raw{widx}")
            t1 = wk.tile([128, T], F32, tag="w1", name=f"t1_{widx}")
            msps = axp.tile([nh, T], F32, tag="aux", name=f"msps{widx}")
            for nchk in range(4):
                cols = ds(512 * nchk, 512)
                ps = pjp.tile([128, 512], F32, tag="pj", name=f"pj{widx}_{nchk}")
                for e in range(8):
                    nc.tensor.matmul(
                        ps[:], w_tiles[e][:, mcols], xsb[e][:, cols],
                        start=(e == 0), stop=(e == 7))
                nc.vector.tensor_copy(_r(raw[:, cols]), ps[:])
                nc.vector.tensor_mul(_r(t1[0:nr, cols]), raw[0:nr, cols],
                                     cos4[0:nr, cols])
            for nchk in range(4):
                cols = ds(512 * nchk, 512)
                sw = swp.tile([128, 512], F32, tag="sw", name=f"sw{widx}_{nchk}")
                nc.tensor.matmul(sw[0:nr, :], _r(cst["psw"][0:nr, 0:nr]),
                                 _r(raw[0:nr, cols]), start=True, stop=True)
                nc.vector.tensor_mul(_r(raw[0:nr, cols]), sw[0:nr, :],
                                     sin4[0:nr, cols])
                nc.vector.tensor_add(_r(raw[0:nr, cols]), raw[0:nr, cols],
                                     t1[0:nr, cols])
                nc.vector.tensor_mul(_r(t1[0:nr, cols]), raw[0:nr, cols],
                                     raw[0:nr, cols])
                nc.tensor.matmul(msps[:, cols], _r(cst["bd"][0:nr, 0:nh]),
                                 _r(t1[0:nr, cols]), start=True, stop=True)
            roped = raw
            # rs = (ms/HD + eps)^-0.5 via Ln then Exp(-0.5 * .): one act
            # each over [nh, T] (partition base 0 -- BIR base-align rule)
            lnm = sm.tile([2, T], F32, tag="lnm", bufs=1, name=f"lnm{widx}")
            nc.scalar.activation(_r(lnm[0:nh, :]), msps[:], AF.Ln,
                                 scale=1.0 / HD, bias=cst["eps"][0:nh, :])
            rs = sm.tile([2, T], F32, tag="rs", bufs=2, name=f"rs{widx}")
            nc.scalar.activation(_r(rs[0:nh, :]), lnm[0:nh, :], AF.Exp,
                                 scale=-0.5)
            return roped, rs

        # kv first (phase B's first scores need kdup + vaug), then q0, q1
        ropedkv, rsk = project_rope(2, wkv_sb, ds(0, 128), 1)
        # kdup: k-hat on both partition halves (for head pairing)
        for nchk in range(4):
            cols = ds(512 * nchk, 512)
            rsbk = swp.tile([128, 512], F32, tag="sw", name=f"rsbk{nchk}")
            nc.tensor.matmul(rsbk[0:64, :], _r(cst["ones64"][:]),
                             _r(rsk[0:1, cols]), start=True, stop=True)
            nc.vector.tensor_mul(_r(kdup[0:64, cols]), ropedkv[0:64, cols],
                                 rsbk[0:64, :])
        nc.sync.dma_start(_r(kdup[64:128, :]), _r(kdup[0:64, :]))
        # vaug: token-major v (+ gate*ve2), ones column appended
        for kb in range(NKB):
            vt = swp.tile([128, HD], F32, tag="sw", name=f"vt{kb}")
            nc.tensor.transpose(vt[:], ropedkv[64:128, ts(kb, 128)],
                                cst["ident"][64:128, 64:128])
            gv = sm.tile([128, HD], F32, tag="gv", bufs=2)
            nc.vector.tensor_scalar_mul(gv[:], ve_sb[:, kb, :],
                                        g2[:, ds(kb, 1)])
            nc.gpsimd.memset(vaug[kb][:, HD:HD + 1], 1.0)
            nc.vector.tensor_add(_r(vaug[kb][:, 0:HD]), gv[:], vt[:])

        # q heads: two tiles of 2 heads each
        for i in range(2):
            roped, rs = project_rope(i, wq_sb, ds(128 * i, 128), 2)
            for nchk in range(4):
                cols = ds(512 * nchk, 512)
                rsb = swp.tile([128, 512], F32, tag="sw",
                               name=f"rsbq{i}_{nchk}")
                nc.tensor.matmul(rsb[:], _r(cst["e2sel"][:]),
                                 _r(rs[0:2, cols]), start=True, stop=True)
                nc.vector.tensor_mul(_r(qhat[i][:, cols]), roped[:, cols],
                                     rsb[:])

    ld["trg"].release()
    ld["xp"].release()
    # next rep's input DMAs are emitted HERE so on each DMA queue they
    # precede this rep's output DMAs (no head-of-line blocking)
    if prefetch is not None:
        prefetch(rep + 1)

    # =================================================================
    # Phase B: attention + output projection (software-pipelined)
    # =================================================================
    with (
        tc.tile_pool(name=f"pb{rep}", bufs=1) as pb,
        tc.tile_pool(name=f"sc_ps{rep}", bufs=2, space="PSUM") as scp,
        tc.tile_pool(name=f"yt_ps{rep}", bufs=2, space="PSUM") as ytp,
        tc.tile_pool(name=f"rb_ps{rep}", bufs=2, space="PSUM") as rbp,
        tc.tile_pool(name=f"et{rep}", bufs=4) as etp,
        tc.tile_pool(name=f"stage{rep}", bufs=4) as stg,
    ):
        ytall = [pb.tile([128, T], F32, tag=f"ytall{i}", name=f"ytall{i}")
                 for i in range(2)]

        def emit_scores(qb, pair, grp, hl):
            w = QB * len(grp)
            if grp[0] == 2 * qb:
                mask, tri = cst["bc01"], cst["triA"]       # diag: causal
            elif grp[0] == 2 * qb - 8:
                mask, tri = cst["bw01"], cst["triA2"]      # window tail
            else:
                mask = tri = None
            sc = scp.tile([128, w], F32, tag="score", bufs=3,
                          name=f"sc{qb}_{pair}_{grp[0]}_{hl}")
            rows = ds(64 * hl, 64)
            qsl = ds(QB * qb, QB)
            for ki, kb in enumerate(grp):
                cols = ds(QB * ki, QB)
                nc.tensor.matmul(
                    sc[:, cols],
                    _r(kdup[rows, ts(kb, 128)]),
                    _r(qhat[pair][rows, qsl]),
                    start=True, stop=(mask is None))
                if mask is not None:
                    # per column-half: the 2KB psum zero region must see
                    # stop before the next half's start
                    nc.tensor.matmul(sc[:, cols], _r(tri[:]),
                                     _r(mask[:, cols]),
                                     start=False, stop=True)
            return sc

        def emit_pv(qb, pair, grp, hl, et, yts, kbs):
            for ki, kb in enumerate(grp):
                nc.tensor.matmul(
                    yts[hl][:], _r(vaug[kb][:]),
                    _r(et[:, ds(QB * ki, QB)]),
                    start=(kb == kbs[0]), stop=(kb == kbs[-1]))

        pending_po = []   # token tiles whose output projection is deferred

        def flush_po():
            for tt in pending_po:
                for nn in range(2):
                    po = rbp.tile([128, 512], F32, tag="rbpo",
                                  name=f"po{tt}_{nn}")
                    for i in range(2):
                        nc.tensor.matmul(
                            po[:], _r(ytall[i][:, ts(tt, 128)]),
                            _r(wo_sb[i][nn][:]),
                            start=(i == 0), stop=(i == 1))
                    osb = stg.tile([128, 512], BF16, tag="osb")
                    nc.vector.tensor_copy(osb[:], po[:])
                    nc.gpsimd.dma_start(
                        out_d[ts(tt, 128), ds(512 * nn, 512)], osb[:])
            pending_po.clear()

        for qb in range(NQB):
            kbs = list(range(max(0, 2 * qb - 8), 2 * qb + 2))
            groups = [kbs[i:i + 2] for i in range(0, len(kbs), 2)]
            for pair in range(2):
                yts = [ytp.tile([HD + 1, QB], F32, tag="yt",
                                name=f"yt{qb}_{pair}_{_}")
                       for _ in range(2)]
                # software pipeline: PV of group g-1 is emitted after the
                # scores of group g, so exp(g-1) overlaps the QK matmuls
                prev = None
                for grp in groups:
                    w = QB * len(grp)
                    scs = [emit_scores(qb, pair, grp, hl) for hl in range(2)]
                    ets = [etp.tile([128, w], F32, tag="et",
                                    name=f"et{qb}_{pair}_{grp[0]}_{_}")
                           for _ in range(2)]
                    for hl in range(2):
                        nc.scalar.activation(_r(ets[hl][:]), scs[hl][:],
                                             AF.Exp, scale=SCALE)
                    if prev is not None:
                        pgrp, pets = prev
                        for hl in range(2):
                            emit_pv(qb, pair, pgrp, hl, pets[hl], yts, kbs)
                    prev = (grp, ets)
                # deferred output projections run while the last exp drains
                if pair == 0 and pending_po:
                    flush_po()
                pgrp, pets = prev
                for hl in range(2):
                    emit_pv(qb, pair, pgrp, hl, pets[hl], yts, kbs)
                for hl in range(2):
                    rinv = sm.tile([1, QB], F32, tag="rinv", bufs=2)
                    nc.vector.reciprocal(_r(rinv[:]), yts[hl][HD:HD + 1, :])
                    rb = rbp.tile([128, 512], F32, tag="rbpo",
                                  name=f"rb{qb}_{pair}_{hl}")
                    nc.tensor.matmul(rb[0:64, 0:QB], _r(cst["ones64"][:]),
                                     _r(rinv[:]), start=True, stop=True)
                    ysb = stg.tile([64, QB], F32, tag="ysb")
                    nc.vector.tensor_copy(ysb[:], yts[hl][0:HD, :])
                    nc.vector.tensor_mul(
                        _r(ytall[pair][ds(64 * hl, 64), ds(QB * qb, QB)]),
                        ysb[:], rb[0:64, 0:QB])
            pending_po.extend((2 * qb, 2 * qb + 1))
        flush_po()


# ---------------------------------------------------------------------------
# walrus workaround: this build rejects >1 sync-wait on CTRL-class ops
# (e.g. the Tile tail Drain). Move excess waits onto NOPs inserted before.
# ---------------------------------------------------------------------------
def _split_excess_waits(nc, limit=1):
    for fn in nc.m.functions:
        for bb in fn.blocks:
            out, changed = [], False
            for inst in bb.instructions:
                si = inst.sync_info
                waits = list(si.on_wait) if si is not None and si.on_wait else []
                if len(waits) > limit:
                    extra, keep = waits[:-limit], waits[-limit:]
                    while extra:
                        chunk, extra = extra[:limit], extra[limit:]
                        nop = mybir.InstNoOp(
                            name=f"{inst.name}-wsplit{len(out)}", ins=[],
                            outs=[])
                        nop.engine = inst.engine
                        nop.sync_info = mybir.SyncInfo(on_wait=chunk,
                                                       on_update=[])
                        out.append(nop)
                    si.on_wait = keep
                    inst.sync_info = si
                    changed = True
                out.append(inst)
            if changed:
                bb.instructions = out


# ---------------------------------------------------------------------------
# Host-side constant block (shared by all cores)
# ---------------------------------------------------------------------------
def _host_constants():
    c = {}
    m = np.arange(128)[:, None]
    j = np.arange(128)[None, :]
    i = np.arange(QB)[None, :]
    c["triA"] = (m <= j).astype(np.float32)          # causal counting lhsT
    c["triA2"] = (m >= j).astype(np.float32)         # window counting lhsT
    bc0 = np.where(m > i, -BIG, 0.0).astype(np.float32)
    bc1 = np.where(m > i - 128, -BIG, 0.0).astype(np.float32)
    bw0 = np.where(m < i, -BIG, 0.0).astype(np.float32)
    bw1 = np.where(m + 128 < i, -BIG, 0.0).astype(np.float32)
    c["bc01"] = np.concatenate([bc0, bc1], axis=1)
    c["bw01"] = np.concatenate([bw0, bw1], axis=1)
    sw = np.zeros((128, 128), np.float32)  # psw[f, m] = +-1 iff f = sig(m)
    for mm in range(128):
        f = mm + 32 if (mm % 64) < 32 else mm - 32
        sw[f, mm] = 1.0 if (mm % 64) < 32 else -1.0
    c["psw"] = sw
    bd = np.zeros((128, 2), np.float32)
    bd[0:64, 0] = 1.0
    bd[64:128, 1] = 1.0
    c["bd"] = bd
    e2 = np.zeros((2, 128), np.float32)
    e2[0, 0:64] = 1.0
    e2[1, 64:128] = 1.0
    c["e2sel"] = e2
    c["ident"] = np.eye(128, dtype=np.float32)
    c["ones64"] = np.ones((1, 64), np.float32)
    return c


def _pack_aux(cstc, cos_b, sin_b, ve_b, Wo_s):
    buf = np.zeros((AUX_LEN,), np.float32)

    def put(nm, arr):
        o = _AUX_OFF[nm]
        buf[o:o + arr.size] = np.ascontiguousarray(arr, np.float32).ravel()

    put("ct", cos_b.T)               # [32, T]
    put("st", sin_b.T)
    put("ve2", 2.0 * ve_b)           # [T, HD] pre-scaled by gate's 2x
    put("wo", Wo_s)
    for nm in ("psw", "bd", "ones64", "e2sel", "bc01", "bw01",
               "triA", "triA2", "ident"):
        put(nm, cstc[nm])
    return buf


def _pack_auxh(Wq_s, Wkv_s, Wg_s):
    import ml_dtypes
    buf = np.zeros((AUXH_LEN,), ml_dtypes.bfloat16)

    def put(nm, arr):
        o = _AUXH_OFF[nm]
        buf[o:o + arr.size] = np.ascontiguousarray(
            arr, np.float32).ravel().astype(ml_dtypes.bfloat16)

    put("wq", Wq_s)
    put("wkv", Wkv_s)
    put("wg", Wg_s)
    return buf


# ---------------------------------------------------------------------------
# Cached PJRT runner (compile once per process)
# ---------------------------------------------------------------------------
_RUNNERS = {}


def _get_runner(nrep=1):
    if nrep in _RUNNERS:
        return _RUNNERS[nrep]
    import jax
    from jax.experimental.shard_map import shard_map
    from jax.sharding import Mesh, PartitionSpec
    from concourse.bass2jax import (_bass_exec_p, install_neuronx_cc_hook,
                                    partition_id_tensor)

    nc = _build_nc(nrep=nrep)
    _split_excess_waits(nc)
    install_neuronx_cc_hook()

    pid_name = (nc.partition_id_tensor.name
                if nc.partition_id_tensor is not None else None)
    in_names, out_names, out_avals, zero_outs = [], [], [], []
    for alloc in nc.m.functions[0].allocations:
        if not isinstance(alloc, mybir.MemoryLocationSet):
            continue
        name = alloc.memorylocations[0].name
        if alloc.kind == "ExternalInput":
            if name == pid_name:
                continue
            in_names.append(name)
        elif alloc.kind == "ExternalOutput":
            np_dt = mybir.dt.np(alloc.dtype)
            out_names.append(name)
            out_avals.append(
                jax.core.ShapedArray(tuple(alloc.tensor_shape), np_dt))
            zero_outs.append(
                np.zeros(tuple(alloc.tensor_shape), np_dt))

    def _body(*args):
        operands = list(args)
        if pid_name is not None:
            operands.append(partition_id_tensor())
        outs = _bass_exec_p.bind(
            *operands,
            out_avals=tuple(out_avals),
            in_names=(tuple(in_names) + tuple(out_names)
                      + ((pid_name,) if pid_name else ())),
            out_names=tuple(out_names),
            lowering_input_output_aliases=(),
            sim_require_finite=True,
            sim_require_nnan=True,
            nc=nc,
        )
        return tuple(outs)

    devices = jax.devices()[:NCORES]
    mesh = Mesh(np.asarray(devices), ("core",))
    n_args = len(in_names) + len(out_names)
    sharded = jax.jit(
        shard_map(_body, mesh=mesh,
                  in_specs=(PartitionSpec("core"),) * n_args,
                  out_specs=(PartitionSpec("core"),) * len(out_names),
                  check_rep=False),
        keep_unused=True,
    )

    def run(in_maps):
        concat_in = [
            np.concatenate([in_maps[c][nm] for c in range(NCORES)], axis=0)
            for nm in in_names
        ]
        concat_zero = [
            np.zeros((NCORES * z.shape[0], *z.shape[1:]), z.dtype)
            for z in zero_outs
        ]
        outs = sharded(*concat_in, *concat_zero)
        res = []
        for c in range(NCORES):
            res.append({
                nm: np.asarray(outs[i]).reshape(NCORES, *out_avals[i].shape)[c]
                for i, nm in enumerate(out_names)
            })
        return res

    _RUNNERS[nrep] = {"run": run, "sharded": sharded, "in_names": in_names,
                      "out_names": out_names, "out_avals": out_avals,
                      "zero_outs": zero_outs, "nc": nc, "mesh": mesh}
    return _RUNNERS[nrep]


def _make_in_maps(x, ve, cos, sin, Wq, Wk, Wv, Wo, Wg):
    cstc = _host_constants()
    in_maps = []
    for c in range(NCORES):
        b, g = c // 4, c % 4
        import ml_dtypes
        aux = _pack_aux(
            cstc, np.asarray(cos[b]), np.asarray(sin[b]),
            np.asarray(ve[b])[:, HD * g:HD * (g + 1)],
            Wo[256 * g:256 * (g + 1), :])
        auxh = _pack_auxh(
            Wq[:, 256 * g:256 * (g + 1)],
            np.concatenate([Wk[:, HD * g:HD * (g + 1)],
                            Wv[:, HD * g:HD * (g + 1)]], axis=1),
            Wg[:, g:g + 1])
        m = {
            "xT": np.ascontiguousarray(np.asarray(x[b]).T).astype(
                ml_dtypes.bfloat16),
            "aux": aux,
            "auxh": auxh,
        }
        in_maps.append(m)
    return in_maps


def kernel(x, ve, cos, sin, Wq, Wk, Wv, Wo, Wg, window_size):
    assert int(window_size) == WIN, f"kernel hardcodes window={WIN}"
    x, ve, cos, sin = (np.asarray(a, np.float32) for a in (x, ve, cos, sin))
    Wq, Wk, Wv, Wo, Wg = (np.asarray(a, np.float32)
                          for a in (Wq, Wk, Wv, Wo, Wg))
    runner = _get_runner()
    in_maps = _make_in_maps(x, ve, cos, sin, Wq, Wk, Wv, Wo, Wg)
    res = runner["run"](in_maps)
    out = np.zeros((B, T, NE), np.float32)
    for c in range(NCORES):
        out[c // 4] += np.asarray(res[c]["out"], np.float32)
    return out


# revision 3
# speedup vs baseline: 294.5735x; 2.0409x over previous
"""Sliding-window causal GQA self-attention kernel for 8 Trainium2 NeuronCores.

v2: engine-balanced rewrite of the baseline.
Sharding: core c -> (batch b = c//4, kv-head g = c%4, q-heads 4g..4g+3).
Each core computes its 4 q-heads' attention and a partial output projection
(y_heads @ Wo[rows]); the host sums the 4 partials per batch.

Changes vs baseline (guided by the CoreSim cost model):
- inputs packed into 2 dram tensors (xT + flat aux) + 1 output -> 3 PJRT
  buffers instead of 35 (each buffer costs ~0.45 ms of axon dispatch wall).
- rmsnorm rsqrt via Ln + Exp(-0.5x) (walrus has no Dsqrt/Rsqrt).
- gate 2*sigmoid via one Sigmoid activation (ve pre-scaled by 2 on host).
- rope sign folded into the signed shuffle matrix psw; trig maps built by
  replicated DMA reads of the 32-row cos/sin tables (2MB instead of 4MB).
- rms/softmax scales broadcast across partitions with tiny K=1
  ones-matmuls into PSUM (walrus rejects GpSimd PartitionBroadcast);
  the following elementwise multiplies read PSUM directly.
- softmax normalization reads PV PSUM directly (no staging copy).
- diag/window mask pairs applied with one [128,512] counting matmul.
- output staged bf16 (halves out DMA); host upcasts and sums partials.
"""

import numpy as np

import concourse.bass as bass
import concourse.mybir as mybir
import concourse.tile as tile
from concourse.bass import ds, ts

F32 = mybir.dt.float32
BF16 = mybir.dt.bfloat16
F32R = mybir.dt.float32r
AF = mybir.ActivationFunctionType

B, T, NE = 2, 2048, 1024
NH, NKV, HD = 16, 4, 64
GC = 32
WIN = 1024
EPS = 1e-6
BIG = 1.0e9
NCORES = 8
QB = 512          # q-block (moving free dim of QK/PV matmuls)
NQB = T // QB     # 4
NKB = T // 128    # 16 k-blocks
SCALE = 1.0 / 8.0  # 1/sqrt(HD)

# ---- flat aux layout: name -> (shape, offset) filled below ----
_AUX_REGIONS = [
    ("ct", (GC, T)), ("st", (GC, T)),
    ("ve2", (T * HD,)),              # host pre-multiplied by 2.0
    ("wo", (256, NE)),
    ("psw", (128, 128)), ("bd", (128, 2)), ("ones64", (1, 64)),
    ("e2sel", (2, 128)),
    ("bc0123", (128, 4 * QB)), ("bw0123", (128, 4 * QB)),
    ("triA", (128, 128)), ("triA2", (128, 128)),
    ("ident", (128, 128)),
]
_AUX_OFF = {}
_off = 0
for _nm, _shp in _AUX_REGIONS:
    _AUX_OFF[_nm] = _off
    _off += int(np.prod(_shp))
AUX_LEN = _off

# bf16 flat input: projection weights (x rides its own bf16 tensor)
_AUXH_REGIONS = [("wq", (NE, 256)), ("wkv", (NE, 128)), ("wg", (GC, 1))]
_AUXH_OFF = {}
_off = 0
for _nm, _shp in _AUXH_REGIONS:
    _AUXH_OFF[_nm] = _off
    _off += int(np.prod(_shp))
AUXH_LEN = _off


def _r(x):
    return x.bitcast(F32R)


def _build_nc(nrep=1):
    nc = bass.Bass(trn_type="TRN2", target_bir_lowering=False)

    xT_d = nc.dram_tensor("xT", [NE, T], BF16, kind="ExternalInput")
    aux_d = nc.dram_tensor("aux", [AUX_LEN], F32, kind="ExternalInput")
    auxh_d = nc.dram_tensor("auxh", [AUXH_LEN], BF16, kind="ExternalInput")
    out_d = nc.dram_tensor("out", [T, NE], BF16, kind="ExternalOutput")

    def aux(nm, p, n):
        """[p, n] view of flat aux region nm."""
        o = _AUX_OFF[nm]
        return aux_d[ds(o, p * n)].rearrange("(p n) -> p n", p=p)

    with tile.TileContext(nc) as tc:
        with (
            nc.allow_low_precision(reason="float32r views; bf16 output"),
            tc.tile_pool(name="persist", bufs=1) as pp,
            tc.tile_pool(name="attbuf", bufs=2) as attp,
            tc.tile_pool(name="smalls", bufs=4) as sm,
        ):
            # ---- persistent constants ----
            cst = {}
            for nm, shp in [("psw", [128, 128]), ("bd", [128, 2]),
                            ("ones64", [1, 64]), ("e2sel", [2, 128]),
                            ("bc0123", [128, 4 * QB]),
                            ("bw0123", [128, 4 * QB]),
                            ("triA", [128, 128]), ("triA2", [128, 128]),
                            ("ident", [128, 128])]:
                cst[nm] = pp.tile(shp, F32, tag=nm, name=f"cst_{nm}")
                nc.sync.dma_start(_r(cst[nm][:]), _r(aux(nm, *shp)))
            cst["wg"] = pp.tile([GC, 1], BF16, tag="wg", name="cst_wg")
            nc.sync.dma_start(
                cst["wg"][:],
                auxh_d[ds(_AUXH_OFF["wg"], GC)].rearrange("(p n) -> p n", p=GC))
            eps_sb = pp.tile([128, 1], F32, tag="eps")
            nc.vector.memset(eps_sb[:], EPS)
            cst["eps"] = eps_sb
            wo_sb = [[pp.tile([128, 512], F32, tag=f"wo{i}{n}", name=f"wo{i}{n}")
                      for n in range(2)] for i in range(2)]
            for i in range(2):
                for n in range(2):
                    nc.sync.dma_start(
                        _r(wo_sb[i][n][:]),
                        _r(aux_d[ds(_AUX_OFF["wo"] + 128 * i * NE, 128 * NE)]
                           .rearrange("(p n) -> p n", p=128)[:, ds(512 * n, 512)]))

            loads = [_load_inputs(nc, tc, xT_d, aux_d, auxh_d, aux, 0,
                                   split_queues=True)]
            for rep in range(nrep):
                _one_rep(nc, tc, attp, sm, aux, out_d, cst, wo_sb, rep,
                         loads[rep],
                         prefetch=(lambda r: loads.append(_load_inputs(
                             nc, tc, xT_d, aux_d, auxh_d, aux, r)))
                         if rep + 1 < nrep else None)
    return nc


def _load_inputs(nc, tc, xT_d, aux_d, auxh_d, aux, rep, split_queues=False):
    """Input DMAs for one rep. All on the SP queue so they are never
    stuck behind output DMAs (Pool queue); rep 0 splits across both
    queues since no outputs are pending yet."""
    xp = tc.alloc_tile_pool(name=f"xp{rep}", bufs=1)
    trg = tc.alloc_tile_pool(name=f"trig{rep}", bufs=1)
    d = {"xp": xp, "trg": trg}
    dq2 = nc.gpsimd if split_queues else nc.sync
    d["xsb"] = [xp.tile([128, T], BF16, tag=f"x{e}", name=f"x{e}_{rep}")
                for e in range(8)]
    d["wq_sb"] = [xp.tile([128, 256], BF16, tag=f"wq{e}", name=f"wqs{e}_{rep}")
                  for e in range(8)]
    d["wkv_sb"] = [xp.tile([128, 128], BF16, tag=f"wkv{e}",
                           name=f"wkvs{e}_{rep}")
                   for e in range(8)]
    for e in range(8):
        dq = dq2 if e % 2 else nc.sync
        dq.dma_start(d["xsb"][e][:], xT_d[ds(128 * e, 128), :])
        nc.sync.dma_start(
            d["wkv_sb"][e][:],
            auxh_d[ds(_AUXH_OFF["wkv"] + 128 * e * 128, 128 * 128)]
            .rearrange("(p n) -> p n", p=128))
        dq.dma_start(
            d["wq_sb"][e][:],
            auxh_d[ds(_AUXH_OFF["wq"] + 128 * e * 256, 128 * 256)]
            .rearrange("(p n) -> p n", p=128))
    d["ve_sb"] = xp.tile([128, NKB, HD], F32, tag="ve", name=f"ve_{rep}")
    nc.sync.dma_start(
        d["ve_sb"][:],
        aux_d[ds(_AUX_OFF["ve2"], T * HD)]
        .rearrange("(n p d) -> p n d", p=128, d=HD))
    # trig maps: 4x replicated 32-row tables, sign carried by psw
    d["cos4"] = trg.tile([128, T], F32, tag="tc", name=f"tc_{rep}")
    d["sin4"] = trg.tile([128, T], F32, tag="tsn", name=f"tsn_{rep}")
    for q in range(4):
        dq = dq2 if q % 2 else nc.sync
        dq.dma_start(d["cos4"][ds(32 * q, 32), :], aux("ct", GC, T))
        dq.dma_start(d["sin4"][ds(32 * q, 32), :], aux("st", GC, T))
    return d


def _one_rep(nc, tc, attp, sm, aux, out_d, cst, wo_sb, rep, ld,
             prefetch=None):
    # cross-rep double-buffered attention tiles (tag reuse rotates bufs)
    qhat = [attp.tile([128, T], F32, tag=f"qhat{i}", name=f"qhat{i}_{rep}")
            for i in range(2)]
    kdup = attp.tile([128, T], F32, tag="kdup", name=f"kdup_{rep}")
    vaug = [attp.tile([128, HD + 1], F32, tag=f"vaug{k}",
                      name=f"vaug{k}_{rep}")
            for k in range(NKB)]
    # =================================================================
    # Phase A: projections + rope + rmsnorm + vaug build
    # =================================================================
    xsb, wq_sb, wkv_sb = ld["xsb"], ld["wq_sb"], ld["wkv_sb"]
    ve_sb, cos4, sin4 = ld["ve_sb"], ld["cos4"], ld["sin4"]
    with (
        tc.tile_pool(name=f"work{rep}", bufs=1) as wk,
        tc.tile_pool(name=f"pj_ps{rep}", bufs=2, space="PSUM") as pjp,
        tc.tile_pool(name=f"sw_ps{rep}", bufs=2, space="PSUM") as swp,
        tc.tile_pool(name=f"aux_ps{rep}", bufs=1, space="PSUM") as axp,
    ):
        # gate: u[t, kb] = x[0:GC, t] @ wg ; g2 = sigmoid(u) (ve carries 2x)
        gate_ps = axp.tile([128, NKB], F32, tag="aux")
        for kb in range(NKB):
            nc.tensor.matmul(
                gate_ps[:, ds(kb, 1)],
                xsb[0][0:GC, ts(kb, 128)], cst["wg"][:],
                start=True, stop=True)
        g2 = ld["xp"].tile([128, NKB], F32, tag="g2", name=f"g2_{rep}")
        nc.scalar.activation(g2[:], gate_ps[:], AF.Sigmoid)

        def project_rope(widx, w_tiles, mcols, nh):
            """x @ W -> [128, T] feature-major roped tile + rms scales.
            Chunked at 512 cols so PE/Pool/DVE stages pipeline.
            Returns (roped [128,T], [nh x [1,T]] rs tiles)."""
            nr = 64 * nh  # rows that get rope+rms (kv: k rows only)
            raw = wk.tile([128, T], F32, tag="w0", bufs=3, name=f"raw{widx}")
            t1 = wk.tile([128, T], F32, tag="w1", name=f"t1_{widx}")
            msps = axp.tile([nh, T], F32, tag="aux", name=f"msps{widx}")
            for nchk in range(4):
                cols = ds(512 * nchk, 512)
                ps = pjp.tile([128, 512], F32, tag="pj", name=f"pj{widx}_{nchk}")
                for e in range(8):
                    nc.tensor.matmul(
                        ps[:], w_tiles[e][:, mcols], xsb[e][:, cols],
                        start=(e == 0), stop=(e == 7))
                nc.vector.tensor_copy(_r(raw[:, cols]), ps[:])
                nc.gpsimd.tensor_mul(_r(t1[0:nr, cols]), raw[0:nr, cols],
                                     cos4[0:nr, cols])
            for nchk in range(4):
                cols = ds(512 * nchk, 512)
                sw = swp.tile([128, 512], F32, tag="sw", name=f"sw{widx}_{nchk}")
                nc.tensor.matmul(sw[0:nr, :], _r(cst["psw"][0:nr, 0:nr]),
                                 _r(raw[0:nr, cols]), start=True, stop=True)
                nc.vector.tensor_mul(_r(raw[0:nr, cols]), sw[0:nr, :],
                                     sin4[0:nr, cols])
                nc.gpsimd.tensor_add(_r(raw[0:nr, cols]), raw[0:nr, cols],
                                     t1[0:nr, cols])
                nc.vector.tensor_mul(_r(t1[0:nr, cols]), raw[0:nr, cols],
                                     raw[0:nr, cols])
                nc.tensor.matmul(msps[:, cols], _r(cst["bd"][0:nr, 0:nh]),
                                 _r(t1[0:nr, cols]), start=True, stop=True)
            roped = raw
            # rs = (ms/HD + eps)^-0.5 via Ln then Exp(-0.5 * .): one act
            # each over [nh, T] (partition base 0 -- BIR base-align rule)
            lnm = sm.tile([2, T], F32, tag="lnm", bufs=1, name=f"lnm{widx}")
            nc.scalar.activation(_r(lnm[0:nh, :]), msps[:], AF.Ln,
                                 scale=1.0 / HD, bias=cst["eps"][0:nh, :])
            rs = sm.tile([2, T], F32, tag="rs", bufs=2, name=f"rs{widx}")
            nc.scalar.activation(_r(rs[0:nh, :]), lnm[0:nh, :], AF.Exp,
                                 scale=-0.5)
            return roped, rs

        # kv first (phase B's first scores need kdup + vaug), then q0, q1
        ropedkv, rsk = project_rope(2, wkv_sb, ds(0, 128), 1)
        # kdup: k-hat on both partition halves (for head pairing)
        for nchk in range(4):
            cols = ds(512 * nchk, 512)
            rsbk = swp.tile([128, 512], F32, tag="sw", name=f"rsbk{nchk}")
            nc.tensor.matmul(rsbk[0:64, :], _r(cst["ones64"][:]),
                             _r(rsk[0:1, cols]), start=True, stop=True)
            nc.vector.tensor_mul(_r(kdup[0:64, cols]), ropedkv[0:64, cols],
                                 rsbk[0:64, :])
        nc.sync.dma_start(_r(kdup[64:128, :]), _r(kdup[0:64, :]))

        # q heads: two tiles of 2 heads each.  The rsb broadcast + qhat
        # multiply of each tile is deferred until after BOTH projections
        # (with the vaug build as extra PE filler) so the Ln/Exp rms
        # tails on the Activation engine never stall the PE.
        ropeds, rss = [], []
        for i in range(2):
            roped, rs = project_rope(i, wq_sb, ds(128 * i, 128), 2)
            ropeds.append(roped)
            rss.append(rs)

        # vaug: token-major v (+ gate*ve2), ones column appended
        for kb in range(NKB):
            vt = swp.tile([128, HD], F32, tag="sw", name=f"vt{kb}")
            nc.tensor.transpose(vt[:], ropedkv[64:128, ts(kb, 128)],
                                cst["ident"][64:128, 64:128])
            gv = sm.tile([128, HD], F32, tag="gv", bufs=2)
            nc.gpsimd.tensor_scalar_mul(gv[:], ve_sb[:, kb, :],
                                        g2[:, ds(kb, 1)])
            nc.gpsimd.memset(vaug[kb][:, HD:HD + 1], 1.0)
            nc.vector.tensor_add(_r(vaug[kb][:, 0:HD]), gv[:], vt[:])

        for i in range(2):
            for nchk in range(4):
                cols = ds(512 * nchk, 512)
                rsb = swp.tile([128, 512], F32, tag="sw",
                               name=f"rsbq{i}_{nchk}")
                nc.tensor.matmul(rsb[:], _r(cst["e2sel"][:]),
                                 _r(rss[i][0:2, cols]), start=True, stop=True)
                nc.vector.tensor_mul(_r(qhat[i][:, cols]),
                                     ropeds[i][:, cols], rsb[:])

    ld["trg"].release()
    ld["xp"].release()
    # next rep's input DMAs are emitted HERE so on each DMA queue they
    # precede this rep's output DMAs (no head-of-line blocking)
    if prefetch is not None:
        prefetch(rep + 1)

    # =================================================================
    # Phase B: attention + output projection (software-pipelined)
    # =================================================================
    with (
        tc.tile_pool(name=f"pb{rep}", bufs=1) as pb,
        tc.tile_pool(name=f"sc_ps{rep}", bufs=2, space="PSUM") as scp,
        tc.tile_pool(name=f"yt_ps{rep}", bufs=2, space="PSUM") as ytp,
        tc.tile_pool(name=f"rb_ps{rep}", bufs=2, space="PSUM") as rbp,
        tc.tile_pool(name=f"et{rep}", bufs=3) as etp,
        tc.tile_pool(name=f"stage{rep}", bufs=4) as stg,
    ):
        ytall = [pb.tile([128, T], F32, tag=f"ytall{i}", name=f"ytall{i}")
                 for i in range(2)]

        def emit_scores(qb, pair, grp, hl):
            w = 512 * len(grp)
            sc = scp.tile([128, w], F32, tag="score", bufs=2,
                          name=f"sc{qb}_{pair}_{grp[0]}_{hl}")
            rows = ds(64 * hl, 64)
            qsl = ds(QB * qb, QB)
            for ki, kb in enumerate(grp):
                d = kb - 4 * qb          # diag offset 0..3 when in-diag
                e = kb - (4 * qb - 8)    # window offset 0..1 when in-tail
                if 0 <= d <= 3:
                    mask, tri = cst["bc0123"][:, ds(512 * d, 512)], cst["triA"]
                elif 0 <= e <= 3:
                    mask, tri = cst["bw0123"][:, ds(512 * e, 512)], cst["triA2"]
                else:
                    mask = tri = None
                cols = ds(512 * ki, 512)
                nc.tensor.matmul(
                    sc[:, cols],
                    _r(kdup[rows, ts(kb, 128)]),
                    _r(qhat[pair][rows, qsl]),
                    start=True, stop=(mask is None))
                if mask is not None:
                    # stop each 2KB psum zero region before the next starts
                    nc.tensor.matmul(sc[:, cols], _r(tri[:]), _r(mask),
                                     start=False, stop=True)
            return sc

        def emit_pv(qb, pair, grp, hl, et, yts, kbs):
            for ki, kb in enumerate(grp):
                nc.tensor.matmul(
                    yts[hl][:], _r(vaug[kb][:]),
                    _r(et[:, ds(512 * ki, 512)]),
                    start=(kb == kbs[0]), stop=(kb == kbs[-1]))

        pending_po = []   # token tiles whose output projection is deferred

        def flush_po():
            for tt in pending_po:
                for nn in range(2):
                    po = rbp.tile([128, 512], F32, tag="rbpo",
                                  name=f"po{tt}_{nn}")
                    for i in range(2):
                        nc.tensor.matmul(
                            po[:], _r(ytall[i][:, ts(tt, 128)]),
                            _r(wo_sb[i][nn][:]),
                            start=(i == 0), stop=(i == 1))
                    osb = stg.tile([128, 512], BF16, tag="osb", bufs=2)
                    nc.vector.tensor_copy(osb[:], po[:])
                    nc.gpsimd.dma_start(
                        out_d[ts(tt, 128), ds(512 * nn, 512)], osb[:])
            pending_po.clear()

        for qb in range(NQB):
            kbs = list(range(max(0, 4 * qb - 8), 4 * qb + 4))
            groups = [kbs[i:i + 2] for i in range(0, len(kbs), 2)]
            for pair in range(2):
                yts = [ytp.tile([HD + 1, 512], F32, tag="yt",
                                name=f"yt{qb}_{pair}_{_}")
                       for _ in range(2)]
                # software pipeline: PV of group g-1 is emitted after the
                # scores of group g, so exp(g-1) overlaps the QK matmuls
                prev = None
                for grp in groups:
                    w = 512 * len(grp)
                    scs = [emit_scores(qb, pair, grp, hl) for hl in range(2)]
                    ets = [etp.tile([128, w], F32, tag="et",
                                    name=f"et{qb}_{pair}_{grp[0]}_{_}")
                           for _ in range(2)]
                    for hl in range(2):
                        nc.scalar.activation(_r(ets[hl][:]), scs[hl][:],
                                             AF.Exp, scale=SCALE)
                    if prev is not None:
                        pgrp, pets = prev
                        for hl in range(2):
                            emit_pv(qb, pair, pgrp, hl, pets[hl], yts, kbs)
                    prev = (grp, ets)
                # deferred output projections run while the last exp drains
                if pair == 0 and pending_po:
                    flush_po()
                pgrp, pets = prev
                for hl in range(2):
                    emit_pv(qb, pair, pgrp, hl, pets[hl], yts, kbs)
                for hl in range(2):
                    rinv = sm.tile([1, 512], F32, tag="rinv", bufs=2)
                    nc.vector.reciprocal(_r(rinv[:]), yts[hl][HD:HD + 1, :])
                    rb = rbp.tile([128, 512], F32, tag="rbpo",
                                  name=f"rb{qb}_{pair}_{hl}")
                    nc.tensor.matmul(rb[0:64, :], _r(cst["ones64"][:]),
                                     _r(rinv[:]), start=True, stop=True)
                    ysb = stg.tile([64, 512], F32, tag="ysb", bufs=2)
                    nc.vector.tensor_copy(ysb[:], yts[hl][0:HD, :])
                    nc.vector.tensor_mul(
                        _r(ytall[pair][ds(64 * hl, 64), ds(QB * qb, QB)]),
                        ysb[:], rb[0:64, :])
            pending_po.extend(range(4 * qb, 4 * qb + 4))
        flush_po()


# ---------------------------------------------------------------------------
# walrus workaround: this build rejects >1 sync-wait on CTRL-class ops
# (e.g. the Tile tail Drain). Move excess waits onto NOPs inserted before.
# ---------------------------------------------------------------------------
def _split_excess_waits(nc, limit=1):
    for fn in nc.m.functions:
        for bb in fn.blocks:
            out, changed = [], False
            for inst in bb.instructions:
                si = inst.sync_info
                waits = list(si.on_wait) if si is not None and si.on_wait else []
                if len(waits) > limit:
                    extra, keep = waits[:-limit], waits[-limit:]
                    while extra:
                        chunk, extra = extra[:limit], extra[limit:]
                        nop = mybir.InstNoOp(
                            name=f"{inst.name}-wsplit{len(out)}", ins=[],
                            outs=[])
                        nop.engine = inst.engine
                        nop.sync_info = mybir.SyncInfo(on_wait=chunk,
                                                       on_update=[])
                        out.append(nop)
                    si.on_wait = keep
                    inst.sync_info = si
                    changed = True
                out.append(inst)
            if changed:
                bb.instructions = out


# ---------------------------------------------------------------------------
# Host-side constant block (shared by all cores)
# ---------------------------------------------------------------------------
def _host_constants():
    c = {}
    m = np.arange(128)[:, None]
    j = np.arange(128)[None, :]
    i = np.arange(QB)[None, :]
    c["triA"] = (m <= j).astype(np.float32)          # causal counting lhsT
    c["triA2"] = (m >= j).astype(np.float32)         # window counting lhsT
    c["bc0123"] = np.concatenate(
        [np.where(m > i - 128 * d, -BIG, 0.0).astype(np.float32)
         for d in range(4)], axis=1)
    c["bw0123"] = np.concatenate(
        [np.where(m + 128 * e < i, -BIG, 0.0).astype(np.float32)
         for e in range(4)], axis=1)
    sw = np.zeros((128, 128), np.float32)  # psw[f, m] = +-1 iff f = sig(m)
    for mm in range(128):
        f = mm + 32 if (mm % 64) < 32 else mm - 32
        sw[f, mm] = 1.0 if (mm % 64) < 32 else -1.0
    c["psw"] = sw
    bd = np.zeros((128, 2), np.float32)
    bd[0:64, 0] = 1.0
    bd[64:128, 1] = 1.0
    c["bd"] = bd
    e2 = np.zeros((2, 128), np.float32)
    e2[0, 0:64] = 1.0
    e2[1, 64:128] = 1.0
    c["e2sel"] = e2
    c["ident"] = np.eye(128, dtype=np.float32)
    c["ones64"] = np.ones((1, 64), np.float32)
    return c


def _pack_aux(cstc, cos_b, sin_b, ve_b, Wo_s):
    buf = np.zeros((AUX_LEN,), np.float32)

    def put(nm, arr):
        o = _AUX_OFF[nm]
        buf[o:o + arr.size] = np.ascontiguousarray(arr, np.float32).ravel()

    put("ct", cos_b.T)               # [32, T]
    put("st", sin_b.T)
    put("ve2", 2.0 * ve_b)           # [T, HD] pre-scaled by gate's 2x
    put("wo", Wo_s)
    for nm in ("psw", "bd", "ones64", "e2sel", "bc0123", "bw0123",
               "triA", "triA2", "ident"):
        put(nm, cstc[nm])
    return buf


def _pack_auxh(Wq_s, Wkv_s, Wg_s):
    import ml_dtypes
    buf = np.zeros((AUXH_LEN,), ml_dtypes.bfloat16)

    def put(nm, arr):
        o = _AUXH_OFF[nm]
        buf[o:o + arr.size] = np.ascontiguousarray(
            arr, np.float32).ravel().astype(ml_dtypes.bfloat16)

    put("wq", Wq_s)
    put("wkv", Wkv_s)
    put("wg", Wg_s)
    return buf


# ---------------------------------------------------------------------------
# Cached PJRT runner (compile once per process)
# ---------------------------------------------------------------------------
_RUNNERS = {}


def _get_runner(nrep=1):
    if nrep in _RUNNERS:
        return _RUNNERS[nrep]
    import jax
    from jax.experimental.shard_map import shard_map
    from jax.sharding import Mesh, PartitionSpec
    from concourse.bass2jax import (_bass_exec_p, install_neuronx_cc_hook,
                                    partition_id_tensor)

    nc = _build_nc(nrep=nrep)
    _split_excess_waits(nc)
    install_neuronx_cc_hook()

    pid_name = (nc.partition_id_tensor.name
                if nc.partition_id_tensor is not None else None)
    in_names, out_names, out_avals, zero_outs = [], [], [], []
    for alloc in nc.m.functions[0].allocations:
        if not isinstance(alloc, mybir.MemoryLocationSet):
            continue
        name = alloc.memorylocations[0].name
        if alloc.kind == "ExternalInput":
            if name == pid_name:
                continue
            in_names.append(name)
        elif alloc.kind == "ExternalOutput":
            np_dt = mybir.dt.np(alloc.dtype)
            out_names.append(name)
            out_avals.append(
                jax.core.ShapedArray(tuple(alloc.tensor_shape), np_dt))
            zero_outs.append(
                np.zeros(tuple(alloc.tensor_shape), np_dt))

    def _body(*args):
        operands = list(args)
        if pid_name is not None:
            operands.append(partition_id_tensor())
        outs = _bass_exec_p.bind(
            *operands,
            out_avals=tuple(out_avals),
            in_names=(tuple(in_names) + tuple(out_names)
                      + ((pid_name,) if pid_name else ())),
            out_names=tuple(out_names),
            lowering_input_output_aliases=(),
            sim_require_finite=True,
            sim_require_nnan=True,
            nc=nc,
        )
        return tuple(outs)

    devices = jax.devices()[:NCORES]
    mesh = Mesh(np.asarray(devices), ("core",))
    n_args = len(in_names) + len(out_names)
    sharded = jax.jit(
        shard_map(_body, mesh=mesh,
                  in_specs=(PartitionSpec("core"),) * n_args,
                  out_specs=(PartitionSpec("core"),) * len(out_names),
                  check_rep=False),
        keep_unused=True,
    )

    def run(in_maps):
        concat_in = [
            np.concatenate([in_maps[c][nm] for c in range(NCORES)], axis=0)
            for nm in in_names
        ]
        concat_zero = [
            np.zeros((NCORES * z.shape[0], *z.shape[1:]), z.dtype)
            for z in zero_outs
        ]
        outs = sharded(*concat_in, *concat_zero)
        res = []
        for c in range(NCORES):
            res.append({
                nm: np.asarray(outs[i]).reshape(NCORES, *out_avals[i].shape)[c]
                for i, nm in enumerate(out_names)
            })
        return res

    _RUNNERS[nrep] = {"run": run, "sharded": sharded, "in_names": in_names,
                      "out_names": out_names, "out_avals": out_avals,
                      "zero_outs": zero_outs, "nc": nc, "mesh": mesh}
    return _RUNNERS[nrep]


def _make_in_maps(x, ve, cos, sin, Wq, Wk, Wv, Wo, Wg):
    cstc = _host_constants()
    in_maps = []
    for c in range(NCORES):
        b, g = c // 4, c % 4
        import ml_dtypes
        aux = _pack_aux(
            cstc, np.asarray(cos[b]), np.asarray(sin[b]),
            np.asarray(ve[b])[:, HD * g:HD * (g + 1)],
            Wo[256 * g:256 * (g + 1), :])
        auxh = _pack_auxh(
            Wq[:, 256 * g:256 * (g + 1)],
            np.concatenate([Wk[:, HD * g:HD * (g + 1)],
                            Wv[:, HD * g:HD * (g + 1)]], axis=1),
            Wg[:, g:g + 1])
        m = {
            "xT": np.ascontiguousarray(np.asarray(x[b]).T).astype(
                ml_dtypes.bfloat16),
            "aux": aux,
            "auxh": auxh,
        }
        in_maps.append(m)
    return in_maps


def kernel(x, ve, cos, sin, Wq, Wk, Wv, Wo, Wg, window_size):
    assert int(window_size) == WIN, f"kernel hardcodes window={WIN}"
    x, ve, cos, sin = (np.asarray(a, np.float32) for a in (x, ve, cos, sin))
    Wq, Wk, Wv, Wo, Wg = (np.asarray(a, np.float32)
                          for a in (Wq, Wk, Wv, Wo, Wg))
    runner = _get_runner()
    in_maps = _make_in_maps(x, ve, cos, sin, Wq, Wk, Wv, Wo, Wg)
    res = runner["run"](in_maps)
    out = np.zeros((B, T, NE), np.float32)
    for c in range(NCORES):
        out[c // 4] += np.asarray(res[c]["out"], np.float32)
    return out
